# revision 1
# baseline (speedup 1.0000x reference)
import sys

sys.path.insert(0, "/opt/trn_rl_repo")

import numpy as np
import ml_dtypes

import concourse.bass as bass
import concourse.mybir as mybir
import concourse.tile as tile
from concourse import bacc
from concourse.bass_utils import run_bass_kernel_spmd

# Problem constants (hardcoded per contract)
N_CORES = 8
B = 32
B_LOC = B // N_CORES  # 4 batches per core
S = 484
E = 1024
H = 1024  # q proj dim = 16 heads * 64
KV = 256  # kv proj dim = 4 groups * 64
G = 4
HKV = 4
NH = 16
D = 64
MD = 484  # MAX_DIST
TW = 2 * MD - 1  # 967 table rows
DW = 968  # bias window width per head
F32 = mybir.dt.float32
F32R = mybir.dt.float32r
BF16 = mybir.dt.bfloat16

# s tiling: 484 = 128*3 + 100
ST = [(0, 128), (128, 128), (256, 128), (384, 100)]
NE = E // 128  # 8 contraction tiles


def _r(ap):
    # operands are declared float32r already
    return ap


def build_nc():
    nc = bacc.Bacc("TRN2", target_bir_lowering=False, debug=False, num_devices=N_CORES)

    xq = nc.dram_tensor("xq", [B_LOC, E, S], F32, kind="ExternalInput")
    xk = nc.dram_tensor("xk", [B_LOC, E, S], F32, kind="ExternalInput")
    xv = nc.dram_tensor("xv", [B_LOC, E, S], F32, kind="ExternalInput")
    wq = nc.dram_tensor("wq", [E, H], F32, kind="ExternalInput")
    wk = nc.dram_tensor("wk", [E, KV], F32, kind="ExternalInput")
    wv = nc.dram_tensor("wv", [E, KV], F32, kind="ExternalInput")
    wo = nc.dram_tensor("wo", [H, E], F32, kind="ExternalInput")
    bd = nc.dram_tensor("bd", [NH, 128, DW], BF16, kind="ExternalInput")
    out = nc.dram_tensor("out", [B_LOC, S, E], F32, kind="ExternalOutput")

    from contextlib import ExitStack

    with tile.TileContext(nc) as tc:
        with ExitStack() as ctx:
            wqp = ctx.enter_context(tc.tile_pool(name="wqp", bufs=1))
            wkp = ctx.enter_context(tc.tile_pool(name="wkp", bufs=1))
            wvp = ctx.enter_context(tc.tile_pool(name="wvp", bufs=1))
            wop = ctx.enter_context(tc.tile_pool(name="wop", bufs=1))
            bdp = ctx.enter_context(tc.tile_pool(name="bdp", bufs=1))
            onep = ctx.enter_context(tc.tile_pool(name="onep", bufs=1))
            xep = ctx.enter_context(tc.tile_pool(name="xe", bufs=4))
            qtp = ctx.enter_context(tc.tile_pool(name="qt", bufs=8))
            kdp = ctx.enter_context(tc.tile_pool(name="kd", bufs=4))
            vhp = ctx.enter_context(tc.tile_pool(name="vh", bufs=4))
            php = ctx.enter_context(tc.tile_pool(name="ph", bufs=2))
            otp = ctx.enter_context(tc.tile_pool(name="ot", bufs=8))
            osp = ctx.enter_context(tc.tile_pool(name="os", bufs=2))
            lvp = ctx.enter_context(tc.tile_pool(name="lv", bufs=2))
            lbp = ctx.enter_context(tc.tile_pool(name="lb", bufs=2))
            psA = ctx.enter_context(tc.tile_pool(name="psA", bufs=6, space="PSUM"))
            psB = ctx.enter_context(tc.tile_pool(name="psB", bufs=2, space="PSUM"))
            # --- resident weights ---
            wq_sb = []
            wk_sb = []
            wv_sb = []
            wo_sb = []
            for e in range(NE):
                t = wqp.tile([128, H], F32R, tag="wq", name="wq_t", bufs=8)
                nc.sync.dma_start(out=t[:], in_=wq[e * 128:(e + 1) * 128, :].bitcast(F32R))
                wq_sb.append(t)
                t = wkp.tile([128, KV], F32R, tag="wk", name="wk_t", bufs=8)
                nc.sync.dma_start(out=t[:], in_=wk[e * 128:(e + 1) * 128, :].bitcast(F32R))
                wk_sb.append(t)
                t = wvp.tile([128, KV], F32R, tag="wv", name="wv_t", bufs=8)
                nc.sync.dma_start(out=t[:], in_=wv[e * 128:(e + 1) * 128, :].bitcast(F32R))
                wv_sb.append(t)
                t = wop.tile([128, E], F32R, tag="wo", name="wo_t", bufs=8)
                nc.sync.dma_start(out=t[:], in_=wo[e * 128:(e + 1) * 128, :].bitcast(F32R))
                wo_sb.append(t)
            # bias windows, one wide bf16 tile
            bd_sb = bdp.tile([128, NH * DW], BF16, tag="bd")
            for h in range(NH):
                nc.sync.dma_start(out=bd_sb[:, h * DW:(h + 1) * DW], in_=bd[h])
            # f32r ones column (ACT rounds f32 -> f32r)
            ones32 = onep.tile([128, 1], F32, tag="ones32", name="ones32")
            nc.vector.memset(ones32[:], 1.0)
            onesr = onep.tile([128, 1], F32R, tag="onesr", name="onesr")
            nc.scalar.copy(onesr[:], ones32[:])

            for b in range(B_LOC):
                # ---------------- K^T and V-hat ----------------
                kps = [psA.tile([128, S], F32, tag="psA", name="psA_t") for _ in range(2)]
                vps = [psA.tile([128, KV], F32, tag="psA", name="psA_v") for _ in range(4)]
                for e in range(NE):
                    xke = xep.tile([128, S], F32R, tag="xe", name="xe_t")
                    nc.sync.dma_start(out=xke[:], in_=xk[b, e * 128:(e + 1) * 128, :].bitcast(F32R))
                    xve = xep.tile([128, S], F32R, tag="xe", name="xe_t")
                    nc.sync.dma_start(out=xve[:], in_=xv[b, e * 128:(e + 1) * 128, :].bitcast(F32R))
                    st = e == 0
                    sp = e == NE - 1
                    for m in range(2):
                        nc.tensor.matmul(
                            kps[m][:],
                            _r(wk_sb[e][:, m * 128:(m + 1) * 128]),
                            _r(xke[:]),
                            start=st,
                            stop=sp,
                        )
                    for si, (s0, sl) in enumerate(ST):
                        nc.tensor.matmul(
                            vps[si][0:sl, :],
                            _r(xve[:, s0:s0 + sl]),
                            _r(wv_sb[e][:]),
                            start=st,
                            stop=sp,
                        )
                # evac K^T into per-group duplicated tiles (group at rows 0-63 AND 64-127)
                kd_sb = [kdp.tile([128, S], F32R, tag="kd", name="kd_t") for _ in range(G)]
                for g in range(G):
                    src = kps[g // 2][(g % 2) * 64:(g % 2) * 64 + 64, :]
                    nc.scalar.copy(kd_sb[g][0:64, :], src)
                    nc.scalar.copy(kd_sb[g][64:128, :], src)
                # evac V into [128, G, 65] tiles with ones column
                vh_sb = []
                for si, (s0, sl) in enumerate(ST):
                    t = vhp.tile([128, G, 65], F32R, tag="vh", name="vh_t")
                    for g in range(G):
                        nc.scalar.copy(t[:, g, :][:, 64:65], onesr[:])
                    nc.scalar.copy(
                        t[0:sl, :, 0:64],
                        vps[si][0:sl, :].rearrange("p (g d) -> p g d", g=G),
                    )
                    vh_sb.append(t)

                # ---------------- Q^T (2 rounds of 4 h-tiles) ----------------
                qt_sb = [qtp.tile([128, S], F32R, tag="qt", name="qt_t") for _ in range(NE)]
                for rnd in range(2):
                    qps = [psA.tile([128, S], F32, tag="psA", name="psA_t") for _ in range(4)]
                    for e in range(NE):
                        xqe = xep.tile([128, S], F32R, tag="xe", name="xe_t")
                        nc.sync.dma_start(
                            out=xqe[:], in_=xq[b, e * 128:(e + 1) * 128, :].bitcast(F32R)
                        )
                        for hi in range(4):
                            ht = rnd * 4 + hi
                            nc.tensor.matmul(
                                qps[hi][:],
                                _r(wq_sb[e][:, ht * 128:(ht + 1) * 128]),
                                _r(xqe[:]),
                                start=(e == 0),
                                stop=(e == NE - 1),
                            )
                    for hi in range(4):
                        nc.vector.tensor_copy(qt_sb[rnd * 4 + hi][:], qps[hi][:])

                # ---------------- attention per head ----------------
                ot_sb = [otp.tile([128, S], F32R, tag="ot", name="ot_t") for _ in range(NE)]
                for hh in range(NH):
                    g = hh // HKV
                    base = (hh % 2) * 64
                    q_ap = qt_sb[hh // 2][base:base + 64, :]
                    p_t = php.tile([128, 4, S], F32R, tag="ph", name="ph_t")
                    for si, (s0, sl) in enumerate(ST):
                        sps = psA.tile([128, S], F32, tag="psA", name="psA_t")
                        nc.tensor.matmul(
                            sps[0:sl, :],
                            _r(kd_sb[g][base:base + 64, s0:s0 + sl]),
                            _r(q_ap),
                            start=True,
                            stop=True,
                        )
                        # p = (s * 0.125 + bias) on DVE, then exp in-place on ACT
                        nc.vector.scalar_tensor_tensor(
                            p_t[0:sl, si, :],
                            sps[0:sl, :],
                            0.125,
                            bd_sb[0:sl, hh * DW + (MD - 1 - s0):hh * DW + (MD - 1 - s0) + S],
                            op0=mybir.AluOpType.mult,
                            op1=mybir.AluOpType.add,
                        )
                        nc.scalar.activation(
                            p_t[0:sl, si, :],
                            p_t[0:sl, si, :],
                            mybir.ActivationFunctionType.Exp,
                        )
                    ops = psB.tile([128, 512], F32, tag="psB", name="psB_t")
                    for si, (s0, sl) in enumerate(ST):
                        nc.tensor.matmul(
                            ops[0:65, 0:S],
                            _r(vh_sb[si][0:sl, g, :]),
                            _r(p_t[0:sl, si, :]),
                            start=(si == 0),
                            stop=(si == 3),
                        )
                    linv = lvp.tile([1, S], F32, tag="lv", name="lv_t")
                    nc.vector.reciprocal(linv[:], ops[64:65, 0:S])
                    lbc = lbp.tile([64, S], F32, tag="lb", name="lb_t")
                    nc.gpsimd.partition_broadcast(lbc[:], linv[:])
                    nc.vector.tensor_mul(
                        ot_sb[hh // 2][base:base + 64, :],
                        ops[0:64, 0:S],
                        lbc[:],
                    )

                # ---------------- output projection ----------------
                for si, (s0, sl) in enumerate(ST):
                    for n in range(2):
                        acc = psB.tile([128, 512], F32, tag="psB", name="psB_t")
                        for dt in range(NE):
                            nc.tensor.matmul(
                                acc[0:sl, :],
                                _r(ot_sb[dt][:, s0:s0 + sl]),
                                _r(wo_sb[dt][:, n * 512:(n + 1) * 512]),
                                start=(dt == 0),
                                stop=(dt == NE - 1),
                            )
                        stg = osp.tile([128, 512], F32, tag="os", name="os_t")
                        nc.scalar.copy(stg[0:sl, :], acc[0:sl, :])
                        nc.sync.dma_start(
                            out=out[b, s0:s0 + sl, n * 512:(n + 1) * 512],
                            in_=stg[0:sl, :],
                        )

    nc.compile()
    return nc


_NC = None


def _get_nc():
    global _NC
    if _NC is None:
        _NC = build_nc()
    return _NC


def _host_prep(query, key, value, Wq, Wk, Wv, Wo, rel_table):
    xq_t = np.ascontiguousarray(query.transpose(0, 2, 1)).astype(np.float32)
    xk_t = np.ascontiguousarray(key.transpose(0, 2, 1)).astype(np.float32)
    xv_t = np.ascontiguousarray(value.transpose(0, 2, 1)).astype(np.float32)
    # bias windows: D[h, i, c] = rel_table[i + 966 - c, h] (0 where out of range)
    ii = np.arange(128)[:, None]
    cc = np.arange(DW)[None, :]
    tidx = ii + (TW - 1) - cc
    valid = (tidx >= 0) & (tidx <= TW - 1)
    tbl = rel_table[np.clip(tidx, 0, TW - 1), :]  # [128, DW, NH]
    tbl = np.where(valid[:, :, None], tbl, 0.0)
    bdv = np.ascontiguousarray(tbl.transpose(2, 0, 1)).astype(ml_dtypes.bfloat16)
    w = {
        "wq": np.ascontiguousarray(Wq, dtype=np.float32),
        "wk": np.ascontiguousarray(Wk, dtype=np.float32),
        "wv": np.ascontiguousarray(Wv, dtype=np.float32),
        "wo": np.ascontiguousarray(Wo, dtype=np.float32),
        "bd": bdv,
    }
    in_maps = []
    for c in range(N_CORES):
        sl = slice(c * B_LOC, (c + 1) * B_LOC)
        in_maps.append(
            {
                "xq": xq_t[sl],
                "xk": xk_t[sl],
                "xv": xv_t[sl],
                **w,
            }
        )
    return in_maps


def _run(inputs, trace=False):
    nc = _get_nc()
    in_maps = _host_prep(**inputs)
    res = run_bass_kernel_spmd(
        nc, in_maps, list(range(N_CORES)), trace=trace
    )
    outp = np.concatenate([r["out"] for r in res.results], axis=0)
    return outp, res


def kernel(query, key, value, Wq, Wk, Wv, Wo, rel_table):
    outp, _ = _run(
        dict(
            query=np.asarray(query),
            key=np.asarray(key),
            value=np.asarray(value),
            Wq=np.asarray(Wq),
            Wk=np.asarray(Wk),
            Wv=np.asarray(Wv),
            Wo=np.asarray(Wo),
            rel_table=np.asarray(rel_table),
        )
    )
    return outp



# revision 76
# speedup vs baseline: 33804.5475x; 33804.5475x over previous
import sys

sys.path.insert(0, "/opt/trn_rl_repo")

import numpy as np
import ml_dtypes

import concourse.bass as bass
import concourse.mybir as mybir
import concourse.tile as tile
from concourse import bacc
from concourse.bass_utils import run_bass_kernel_spmd

# Problem constants (hardcoded per contract)
N_CORES = 8
B = 32
B_LOC = B // N_CORES  # 4 batches per core
S = 484
E = 1024
H = 1024  # q proj dim = 16 heads * 64
KV = 256  # kv proj dim = 4 groups * 64
G = 4
HKV = 4
NH = 16
D = 64
MD = 484  # MAX_DIST
TW = 2 * MD - 1  # 967 table rows
DW = 968  # bias window width per head (4B-aligned slice offsets)
F32 = mybir.dt.float32
F32R = mybir.dt.float32r
BF16 = mybir.dt.bfloat16

# s tiling: 484 = 128*3 + 100
ST = [(0, 128), (128, 128), (256, 128), (384, 100)]
NE = E // 128  # 8 contraction tiles


def build_nc():
    nc = bacc.Bacc("TRN2", target_bir_lowering=False, debug=False, num_devices=N_CORES)

    # x inputs pre-transposed to [b, E, S] and pre-cast to bf16 on host
    xq = nc.dram_tensor("xq", [B_LOC, E, S], BF16, kind="ExternalInput")
    xk = nc.dram_tensor("xk", [B_LOC, E, S], BF16, kind="ExternalInput")
    xv = nc.dram_tensor("xv", [B_LOC, E, S], BF16, kind="ExternalInput")
    wq = nc.dram_tensor("wq", [E, H], BF16, kind="ExternalInput")  # pre-scaled by 1/8
    wk = nc.dram_tensor("wk", [E, KV], BF16, kind="ExternalInput")
    wv = nc.dram_tensor("wv", [E, KV], BF16, kind="ExternalInput")
    wo = nc.dram_tensor("wo", [H, E], BF16, kind="ExternalInput")
    bd = nc.dram_tensor("bd", [NH, 128, DW], BF16, kind="ExternalInput")
    idn = nc.dram_tensor("idn", [128, 128], BF16, kind="ExternalInput")
    sel = nc.dram_tensor("sel", [2, 128, 64], BF16, kind="ExternalInput")
    out = nc.dram_tensor("out", [B_LOC, S, E], F32, kind="ExternalOutput")

    from contextlib import ExitStack

    with tile.TileContext(nc) as tc:
        with ExitStack() as ctx:
            wqp = ctx.enter_context(tc.tile_pool(name="wqp", bufs=1))
            wkp = ctx.enter_context(tc.tile_pool(name="wkp", bufs=1))
            wvp = ctx.enter_context(tc.tile_pool(name="wvp", bufs=1))
            wop = ctx.enter_context(tc.tile_pool(name="wop", bufs=1))
            bdp = ctx.enter_context(tc.tile_pool(name="bdp", bufs=1))
            onep = ctx.enter_context(tc.tile_pool(name="onep", bufs=1))
            xep = ctx.enter_context(tc.tile_pool(name="xe", bufs=4))
            qtp = ctx.enter_context(tc.tile_pool(name="qt", bufs=16))
            kdp = ctx.enter_context(tc.tile_pool(name="kd", bufs=8))
            vhp = ctx.enter_context(tc.tile_pool(name="vh", bufs=8))
            php = ctx.enter_context(tc.tile_pool(name="ph", bufs=4))
            otp = ctx.enter_context(tc.tile_pool(name="ot", bufs=16))
            lvp = ctx.enter_context(tc.tile_pool(name="lv", bufs=8))
            osp = ctx.enter_context(tc.tile_pool(name="os", bufs=2))
            psA = ctx.enter_context(tc.tile_pool(name="psA", bufs=6, space="PSUM"))
            psB = ctx.enter_context(tc.tile_pool(name="psB", bufs=2, space="PSUM"))

            # --- resident weights (bf16), DMA-ordered by first use ---
            # 1. batch-0 xk first half, then K weights, then xv/V weights
            xk0_sb = xep.tile([128, NE, S], BF16, tag="xe", name="xk_t")
            xv0_sb = xep.tile([128, NE, S], BF16, tag="xe", name="xv_t")
            nc.sync.dma_start(
                out=xk0_sb[:, 0:4, :],
                in_=xk[0].rearrange("(e p) s -> p e s", p=128)[:, 0:4, :],
            )
            wk_sb = []
            wv_sb = []
            for e in range(NE):
                t = wkp.tile([128, KV], BF16, tag="wk", name="wk_t", bufs=8)
                nc.sync.dma_start(out=t[:], in_=wk[e * 128:(e + 1) * 128, :])
                wk_sb.append(t)
            nc.sync.dma_start(
                out=xv0_sb[:, 0:4, :],
                in_=xv[0].rearrange("(e p) s -> p e s", p=128)[:, 0:4, :],
            )
            for e in range(NE):
                t = wvp.tile([128, KV], BF16, tag="wv", name="wv_t", bufs=8)
                nc.sync.dma_start(out=t[:], in_=wv[e * 128:(e + 1) * 128, :])
                wv_sb.append(t)
            nc.sync.dma_start(
                out=xk0_sb[:, 4:8, :],
                in_=xk[0].rearrange("(e p) s -> p e s", p=128)[:, 4:8, :],
            )
            nc.sync.dma_start(
                out=xv0_sb[:, 4:8, :],
                in_=xv[0].rearrange("(e p) s -> p e s", p=128)[:, 4:8, :],
            )
            # 3. Q weights + batch-0 xq (interleaved: first Q matmul needs
            # wq[0] + xq0 first half only)
            wq_sb = []
            xq0_sb = xep.tile([128, NE, S], BF16, tag="xe", name="xq_t")
            for e in range(2):
                t = wqp.tile([128, H], BF16, tag="wq", name="wq_t", bufs=8)
                nc.sync.dma_start(out=t[:], in_=wq[e * 128:(e + 1) * 128, :])
                wq_sb.append(t)
            nc.sync.dma_start(
                out=xq0_sb[:, 0:4, :],
                in_=xq[0].rearrange("(e p) s -> p e s", p=128)[:, 0:4, :],
            )
            for e in range(2, NE):
                t = wqp.tile([128, H], BF16, tag="wq", name="wq_t", bufs=8)
                nc.sync.dma_start(out=t[:], in_=wq[e * 128:(e + 1) * 128, :])
                wq_sb.append(t)
            nc.sync.dma_start(
                out=xq0_sb[:, 4:8, :],
                in_=xq[0].rearrange("(e p) s -> p e s", p=128)[:, 4:8, :],
            )
            # 4. attention-time constants: bias windows + identity + selectors
            bd_sb = bdp.tile([128, NH * DW], BF16, tag="bd")
            for h in range(NH):
                nc.sync.dma_start(out=bd_sb[:, h * DW:(h + 1) * DW], in_=bd[h])
            id_sb = onep.tile([128, 128], BF16, tag="idn", name="idn")
            nc.sync.dma_start(out=id_sb[:], in_=idn[:, :])
            # 5. out-projection weights (needed last)
            wo_sb = []
            for e in range(NE):
                t = wop.tile([128, E], BF16, tag="wo", name="wo_t", bufs=8)
                nc.sync.dma_start(out=t[:], in_=wo[e * 128:(e + 1) * 128, :])
                wo_sb.append(t)
            # bf16 ones block for the vh ones-columns
            ones4 = onep.tile([128, 4, 1], BF16, tag="ones4", name="ones4")
            nc.vector.memset(ones4[:], 1.0)
            # f32r row-selector matrices (ones in row 0 / row 32) for rank-1
            # denominator broadcasts via PE
            sel_sb = [onep.tile([128, 64], BF16, tag="sel", name="sel_t", bufs=2) for _ in range(2)]
            for i in range(2):
                nc.sync.dma_start(out=sel_sb[i][:], in_=sel[i])

            pending_out = []

            def emit_outproj(b, ot_sb):
                for si, (s0, sl) in enumerate(ST):
                    for n in range(2):
                        acc = psB.tile([128, 512], F32, tag="psB", name="psB_t")
                        for dt in range(NE):
                            nc.tensor.matmul(
                                acc[0:sl, :],
                                ot_sb[dt][:, s0:s0 + sl],
                                wo_sb[dt][:, n * 512:(n + 1) * 512],
                                start=(dt == 0),
                                stop=(dt == NE - 1),
                            )
                        stg = osp.tile([128, 512], F32, tag="os", name="os_t")
                        nc.vector.tensor_copy(stg[0:sl, :], acc[0:sl, :])
                        nc.sync.dma_start(
                            out=out[b, s0:s0 + sl, n * 512:(n + 1) * 512],
                            in_=stg[0:sl, :],
                        )

            for b in range(B_LOC):
                # ---------------- K^T and V-hat ----------------
                if b == 0:
                    xk_sb, xv_sb = xk0_sb, xv0_sb
                else:
                    xk_sb = xep.tile([128, NE, S], BF16, tag="xe", name="xk_t")
                    for half in range(2):
                        nc.sync.dma_start(
                            out=xk_sb[:, 4 * half:4 * half + 4, :],
                            in_=xk[b].rearrange("(e p) s -> p e s", p=128)[:, 4 * half:4 * half + 4, :],
                        )
                    xv_sb = xep.tile([128, NE, S], BF16, tag="xe", name="xv_t")
                    for half in range(2):
                        nc.sync.dma_start(
                            out=xv_sb[:, 4 * half:4 * half + 4, :],
                            in_=xv[b].rearrange("(e p) s -> p e s", p=128)[:, 4 * half:4 * half + 4, :],
                        )
                kps = [psA.tile([128, S], F32, tag="psA", name="psA_k") for _ in range(2)]
                vps = [psA.tile([128, KV], F32, tag="psA", name="psA_v") for _ in range(4)]
                for e in range(NE):
                    st = e == 0
                    sp = e == NE - 1
                    for m in range(2):
                        nc.tensor.matmul(
                            kps[m][:],
                            wk_sb[e][:, m * 128:(m + 1) * 128],
                            xk_sb[:, e, :],
                            start=st,
                            stop=sp,
                        )
                    for si, (s0, sl) in enumerate(ST):
                        nc.tensor.matmul(
                            vps[si][0:sl, :],
                            xv_sb[:, e, s0:s0 + sl],
                            wv_sb[e][:],
                            start=st,
                            stop=sp,
                        )
                # evac K^T into per-group duplicated bf16 tiles (rows 0-63 AND 64-127)
                kd_sb = [kdp.tile([128, S], BF16, tag="kd", name="kd_t") for _ in range(G)]
                for g in range(G):
                    src = kps[g // 2][(g % 2) * 64:(g % 2) * 64 + 64, :]
                    nc.vector.tensor_copy(kd_sb[g][0:64, :], src)
                    nc.vector.tensor_copy(kd_sb[g][64:128, :], src)
                # evac V into [128, G, 65] bf16 tiles with ones column
                vh_sb = []
                for si, (s0, sl) in enumerate(ST):
                    t = vhp.tile([128, G, 65], BF16, tag="vh", name="vh_t")
                    nc.scalar.copy(t[:, :, 64:65], ones4[:])
                    nc.vector.tensor_copy(
                        t[0:sl, :, 0:64],
                        vps[si][0:sl, :].rearrange("p (g d) -> p g d", g=G),
                    )
                    vh_sb.append(t)

                # ---------------- Q^T (2 rounds of 4 h-tiles) ----------------
                if b == 0:
                    xq_sb = xq0_sb
                else:
                    xq_sb = xep.tile([128, NE, S], BF16, tag="xe", name="xq_t")
                    for half in range(2):
                        nc.sync.dma_start(
                            out=xq_sb[:, 4 * half:4 * half + 4, :],
                            in_=xq[b].rearrange("(e p) s -> p e s", p=128)[:, 4 * half:4 * half + 4, :],
                        )
                qt_sb = [qtp.tile([128, S], BF16, tag="qt", name="qt_t") for _ in range(NE)]
                for rnd in range(2):
                    qps = [psA.tile([128, S], F32, tag="psA", name="psA_q") for _ in range(4)]
                    for e in range(NE):
                        for hi in range(4):
                            ht = rnd * 4 + hi
                            nc.tensor.matmul(
                                qps[hi][:],
                                wq_sb[e][:, ht * 128:(ht + 1) * 128],
                                xq_sb[:, e, :],
                                start=(e == 0),
                                stop=(e == NE - 1),
                            )
                    for hi in range(4):
                        nc.vector.tensor_copy(qt_sb[rnd * 4 + hi][:], qps[hi][:])

                # flush previous batch's deferred out-projection here, after
                # this batch's K/V/Q matmuls are queued on the PE
                while pending_out:
                    emit_outproj(*pending_out.pop(0))

                # ---------------- attention per head ----------------
                ot_sb = [otp.tile([128, S], BF16, tag="ot", name="ot_t") for _ in range(NE)]
                # 4 collectors; head hh -> collector hh//4, partition 32*(hh%4)
                den_c = [lvp.tile([128, S], F32, tag="lv", name="den_t") for _ in range(4)]
                for c in range(4):
                    nc.vector.memset(den_c[c][:], 1.0)
                def emit_scores(hh):
                    g = hh // HKV
                    base = (hh % 2) * 64
                    q_ap = qt_sb[hh // 2][base:base + 64, :]
                    p_t = php.tile([128, 4, S], BF16, tag="ph", name="ph_t")
                    sps_l = []
                    for si, (s0, sl) in enumerate(ST):
                        sps = psA.tile([128, S], F32, tag="psA", name="psA_s")
                        off = hh * DW + (MD - s0)
                        nc.tensor.matmul(
                            sps[0:sl, :],
                            id_sb[0:sl, 0:sl],
                            bd_sb[0:sl, off:off + S],
                            start=True,
                            stop=False,
                        )
                        sps_l.append(sps)
                    for si, (s0, sl) in enumerate(ST):
                        sps = sps_l[si]
                        nc.tensor.matmul(
                            sps[0:sl, :],
                            kd_sb[g][base:base + 64, s0:s0 + sl],
                            q_ap,
                            start=False,
                            stop=True,
                        )
                        nc.scalar.activation(
                            p_t[0:sl, si, :],
                            sps[0:sl, :],
                            mybir.ActivationFunctionType.Exp,
                        )
                    return p_t

                def emit_pv(hh, p_t):
                    g = hh // HKV
                    base = (hh % 2) * 64
                    ops = psB.tile([128, 512], F32, tag="psB", name="psB_t")
                    for si, (s0, sl) in enumerate(ST):
                        nc.tensor.matmul(
                            ops[0:65, 0:S],
                            vh_sb[si][0:sl, g, :],
                            p_t[0:sl, si, :],
                            start=(si == 0),
                            stop=(si == 3),
                        )
                    # evac unnormalized numerator + denominator row; frees PSUM fast
                    nc.scalar.copy(ot_sb[hh // 2][base:base + 64, :], ops[0:64, 0:S])
                    dpo = 32 * (hh % 4)
                    nc.vector.tensor_copy(
                        den_c[hh // 4][dpo:dpo + 1, :], ops[64:65, 0:S]
                    )

                # software-pipelined: scores of head hh+1 issue before PV of hh
                prev = None
                for hh in range(NH):
                    p_t = emit_scores(hh)
                    if prev is not None:
                        emit_pv(prev[0], prev[1])
                    prev = (hh, p_t)
                emit_pv(prev[0], prev[1])
                # batched reciprocal per collector, then per-pair bcast+scale
                dinv_c = []
                for c in range(4):
                    dv = lvp.tile([128, S], BF16, tag="lv", name="dinv_t")
                    with nc.allow_low_precision(reason="bf16 dinv for rank-1 bcast"):
                        nc.vector.reciprocal(dv[:], den_c[c][:])
                    dinv_c.append(dv)
                for r in range(NE):
                    # heads 2r, 2r+1 -> collector r//2, partitions 64*(r%2), +32
                    dv = dinv_c[r // 2]
                    po = 64 * (r % 2)
                    # selector-matmul broadcast over the 64-row block at po:
                    # lps[0:64] = dinv row po (head 2r), lps[64:128] = row po+32
                    lps = psA.tile([128, S], F32, tag="psA", name="psA_l")
                    nc.tensor.matmul(
                        lps[0:64, :],
                        sel_sb[0][po:po + 64, :],
                        dv[po:po + 64, :],
                        start=True,
                        stop=True,
                        tile_position=(po, 0),
                    )
                    nc.tensor.matmul(
                        lps[64:128, :],
                        sel_sb[1][po:po + 64, :],
                        dv[po:po + 64, :],
                        start=True,
                        stop=True,
                        tile_position=(po, 64),
                    )
                    nc.vector.tensor_mul(ot_sb[r][:], ot_sb[r][:], lps[:])

                # defer out-projection of this batch until after the next
                # batch's K/V/Q matmuls (keeps PE from stalling on the muls)
                pending_out.append((b, ot_sb))

            while pending_out:
                emit_outproj(*pending_out.pop(0))

    nc.compile()
    return nc


_NC = None


def _get_nc():
    global _NC
    if _NC is None:
        _NC = build_nc()
    return _NC


def _host_prep(query, key, value, Wq, Wk, Wv, Wo, rel_table):
    bf = ml_dtypes.bfloat16
    xq_t = np.ascontiguousarray(query.transpose(0, 2, 1)).astype(bf)
    xk_t = np.ascontiguousarray(key.transpose(0, 2, 1)).astype(bf)
    xv_t = np.ascontiguousarray(value.transpose(0, 2, 1)).astype(bf)
    # bias windows: D[h, i, c] = rel_table[i + (TW) - c, h] (0 where out of range)
    # CONST=967 so slice offsets (MD - s0) are 4B-aligned for the fp8 matmul operand
    ii = np.arange(128)[:, None]
    cc = np.arange(DW)[None, :]
    tidx = ii + TW - cc
    valid = (tidx >= 0) & (tidx <= TW - 1)
    tbl = np.asarray(rel_table)[np.clip(tidx, 0, TW - 1), :]  # [128, DW, NH]
    tbl = np.where(valid[:, :, None], tbl, 0.0)
    bdv = np.ascontiguousarray(tbl.transpose(2, 0, 1)).astype(bf)
    selv = np.zeros((2, 128, 64), dtype=np.float32)
    selv[0, 0, :] = 1.0
    selv[0, 64, :] = 1.0
    selv[1, 32, :] = 1.0
    selv[1, 96, :] = 1.0
    selv = selv.astype(bf)
    w = {
        "wq": (np.asarray(Wq, dtype=np.float32) * 0.125).astype(bf),
        "wk": np.asarray(Wk).astype(bf),
        "wv": np.asarray(Wv).astype(bf),
        "wo": np.asarray(Wo).astype(bf),
        "bd": bdv,
        "idn": np.eye(128, dtype=np.float32).astype(bf),
        "sel": selv,
    }
    in_maps = []
    for c in range(N_CORES):
        sl = slice(c * B_LOC, (c + 1) * B_LOC)
        in_maps.append(
            {
                "xq": xq_t[sl],
                "xk": xk_t[sl],
                "xv": xv_t[sl],
                **w,
            }
        )
    return in_maps


def _run(inputs, trace=False):
    nc = _get_nc()
    in_maps = _host_prep(**inputs)
    res = run_bass_kernel_spmd(
        nc, in_maps, list(range(N_CORES)), trace=trace
    )
    outp = np.concatenate([r["out"] for r in res.results], axis=0)
    return outp, res


def kernel(query, key, value, Wq, Wk, Wv, Wo, rel_table):
    outp, _ = _run(
        dict(
            query=np.asarray(query),
            key=np.asarray(key),
            value=np.asarray(value),
            Wq=np.asarray(Wq),
            Wk=np.asarray(Wk),
            Wv=np.asarray(Wv),
            Wo=np.asarray(Wo),
            rel_table=np.asarray(rel_table),
        )
    )
    return outp
